# revision 29
# baseline (speedup 1.0000x reference)
"""DescriptorLoss kernel for Trainium2 (8 NeuronCores, SPMD data-parallel).

Math (d' = 5*d, hinges at d'=1 (neg branch, m=0) and d'=5 (pos branch, m=1)):
    loss*5*N = sum_{m=0} relu(d'-1) + 250 * sum_{m=1} relu(5-d')

Per core: shard = (batch, 16-row i-slab) -> 1024 ij rows x 4096 kl cols,
8 groups (128 rows) x 2 pairs (2048 cols) = 16 pair-tiles.

Identity: with dM = d' - 1024*m (PE-injected mask offset), t1 = dM - 1,
u = |t1|:
  - m=0: u = |d'-1| <= ~510;  m=1: u = 1025-d' in [~515, ~1535]
    (ranges separated; |d'| < 9 sigma ~ 510).
  - hinge1 = sum relu(t1) = 0.5*(sum t1 + sum u); sum t1 is linear
    (rank-1 a.b sums + mask popcount) -> host f64.
  - hinge2 = sum relu(u-1020) = sum u - sum min(u, 1020).

Pipeline per pair [128 x 2048]:
  PE:  4 mains (a5 stationary) + 4 injects (idn=-I stationary, mask 0/1024
       fp8 moving) -> PSUM fp32 dM.
  ACT: activation(Abs, bias=-1) PSUM->SBUF fp16 u-tile + accum_out = sum u.
  DVE: per group, one tensor_scalar (min 1020, add-reduce) FD=4096 over the
       u-tile -> sum min(u, 1020).
Host combines in f64; loss = total / (5*B*IJ^2).
"""

import numpy as np
import ml_dtypes

import concourse.bacc as bacc
import concourse.mybir as mybir
import concourse.tile as tile
from concourse.bass_utils import run_bass_kernel_spmd

B, D, H, W = 2, 128, 64, 64
N_CORES = 8
IJ = H * W               # 4096
ROWS = IJ // 4           # 1024 rows per core
G = ROWS // 128          # 8 row groups
PAIR = 2048              # egress tile width
N_PAIRS = G * 2          # 16
OMEGA = 1024.0
TH = OMEGA - 4.0         # 1020
GPS_MIN_PAIRS = ()       # TensorScalarPtr reduce unsupported on Pool engine
MMW = 512                # matmul moving-operand width (PSUM bank limit)

_cached = {}


def _build_program():
    nc = bacc.Bacc("TRN2")
    f32 = mybir.dt.float32
    bf16 = mybir.dt.bfloat16
    f16 = mybir.dt.float16
    f8 = mybir.dt.float8e5
    f8e4 = mybir.dt.float8e4
    Alu = mybir.AluOpType
    Act = mybir.ActivationFunctionType

    a5 = nc.declare_dram_parameter("a5", [D, ROWS], bf16, isOutput=False)
    bm = nc.declare_dram_parameter("bm", [D, IJ], f8e4, isOutput=False)
    m8 = nc.declare_dram_parameter("m8", [ROWS, IJ], f8, isOutput=False)
    idn = nc.declare_dram_parameter("idn", [D, D], bf16, isOutput=False)
    accs_out = nc.declare_dram_parameter(
        "accs", [128, 2 * N_PAIRS + 8], f32, isOutput=True)

    # unit = (group g, column-pair p), processed column-major: all groups at
    # p=0 first (needs only b[:, :2048]), then p=1 — halves the DMA ramp.
    units = [(g, p) for p in range(2) for g in range(G)]
    SPECIAL = len(units) - 1   # last unit: 2nd ACT Abs pass instead of min

    with tile.TileContext(nc) as tc:
        with (
            tc.tile_pool(name="desc", bufs=1) as desc_pool,
            tc.tile_pool(name="mask", bufs=5) as mask_pool,
            tc.tile_pool(name="t1", bufs=3) as t1_pool,
            tc.tile_pool(name="junk", bufs=2) as junk_pool,
            tc.tile_pool(name="acc", bufs=1) as acc_pool,
            tc.tile_pool(name="ps", bufs=2, space="PSUM") as ps_pool,
        ):
            a_t = desc_pool.tile([D, ROWS], bf16, tag="a")
            b_t = desc_pool.tile([D, IJ], f8e4, tag="b")
            id_t = desc_pool.tile([D, D], bf16, tag="idn")
            bias_t = desc_pool.tile([128, 1], f32, tag="bias")
            bias2_t = desc_pool.tile([128, 1], f32, tag="bias2")
            prime_t = desc_pool.tile([128, 1], f16, tag="prime")
            accAll = acc_pool.tile([128, 2 * N_PAIRS + 8], f32,
                                   tag="accAll")

            nc.gpsimd.memset(bias_t[:], -1.0)
            nc.gpsimd.memset(bias2_t[:], TH - 1.0)
            # Prime the ACT table set (Abs): ~2.7us load overlaps early DMAs.
            nc.scalar.activation(prime_t[:], bias_t[:], Act.Abs,
                                 bias=bias_t[:], scale=1.0)

            m_tiles = {}

            def load_mask(g, p, eng):
                mt = mask_pool.tile([128, PAIR], f8, tag="m8")
                rs = slice(g * 128, (g + 1) * 128)
                ks = slice(p * PAIR, (p + 1) * PAIR)
                eng.dma_start(mt[:], m8[rs, ks])
                m_tiles[(g, p)] = mt

            # ramp-critical loads, split across the two HWDGE queues
            nc.sync.dma_start(a_t[:, :128], a5[:, :128])
            nc.sync.dma_start(b_t[:, :512], bm[:, :512])
            nc.sync.dma_start(b_t[:, 512:1024], bm[:, 512:1024])
            load_mask(0, 0, nc.scalar)
            nc.scalar.dma_start(id_t[:], idn[:])
            nc.scalar.dma_start(b_t[:, 1024:PAIR], bm[:, 1024:PAIR])
            load_mask(1, 0, nc.sync)
            nc.scalar.dma_start(a_t[:, 128:], a5[:, 128:])
            load_mask(2, 0, nc.sync)

            min_queue = []  # (aid, tile, ncols) pending min-ops

            def do_min(col, pt, csl):
                jk = junk_pool.tile([128, PAIR], f16, tag="junk")
                nc.vector.tensor_scalar(
                    jk[:, :csl.stop - csl.start], pt[:, csl], TH, 0.0,
                    op0=Alu.min, op1=Alu.add,
                    accum_out=accAll[:, col:col + 1],
                )

            for ui, (g, p) in enumerate(units):
                rs = slice(g * 128, (g + 1) * 128)
                t1_t = t1_pool.tile([128, PAIR], f16, tag="t1")
                half = 0
                pid = g * 2 + p

                pst = ps_pool.tile([128, PAIR], f32, tag="d")
                mt = m_tiles[(g, p)]
                for h in range(PAIR // MMW):
                    hs = slice(h * MMW, (h + 1) * MMW)
                    cs = slice(p * PAIR + h * MMW, p * PAIR + (h + 1) * MMW)
                    nc.tensor.matmul(pst[:, hs], a_t[:, rs], b_t[:, cs],
                                     start=True, stop=False)

                # prefetch: masks two units ahead; b second half mid-phase-0
                if ui + 3 < len(units):
                    ng, np_ = units[ui + 3]
                    load_mask(ng, np_, nc.sync)
                if ui == 2:
                    nc.scalar.dma_start(b_t[:, PAIR:3072], bm[:, PAIR:3072])
                if ui == 4:
                    nc.sync.dma_start(b_t[:, 3072:], bm[:, 3072:])

                for h in range(PAIR // MMW):
                    hs = slice(h * MMW, (h + 1) * MMW)
                    nc.tensor.matmul(pst[:, hs], id_t[:], mt[:, hs],
                                     start=False, stop=True)

                # ACT egress: u = |dM - 1| fp16 + accum(sum u)
                nc.scalar.activation(
                    t1_t[:, half:half + PAIR], pst[:],
                    Act.Abs, bias=bias_t[:], scale=1.0,
                    accum_out=accAll[:, pid:pid + 1],
                )
                if ui == SPECIAL:
                    # second Abs pass on the same PSUM: A2 = sum|dM + 1019|
                    # (two-abs identity supplies this unit's hinges; no min)
                    jk2 = junk_pool.tile([128, PAIR], f16, tag="junk")
                    nc.scalar.activation(
                        jk2[:, :PAIR], pst[:],
                        Act.Abs, bias=bias2_t[:], scale=1.0,
                        accum_out=accAll[:, N_PAIRS + ui:N_PAIRS + ui + 1],
                    )
                else:
                    min_queue.append((N_PAIRS + ui, t1_t, slice(0, PAIR)))

                while len(min_queue) > 1:
                    do_min(*min_queue.pop(0))

            while min_queue:
                do_min(*min_queue.pop(0))

            nc.sync.dma_start(accs_out[:, :N_PAIRS], accAll[:, :N_PAIRS])
            nc.sync.dma_start(accs_out[:, N_PAIRS:], accAll[:, N_PAIRS:])

    nc.finalize()
    return nc


def _prep_inputs(descriptors_0, descriptors_1, similarity_mask):
    d0 = np.asarray(descriptors_0, dtype=np.float32)
    d1 = np.asarray(descriptors_1, dtype=np.float32)
    mkv = np.asarray(similarity_mask)
    idn = (-np.eye(D, dtype=np.float32)).astype(ml_dtypes.bfloat16)
    in_maps = []
    side = []
    for c in range(N_CORES):
        b = c >> 2
        isl = (c & 3) * 16
        a5 = (d0[b].reshape(D, IJ)[:, isl * W:(isl + 16) * W]
              * np.float32(5.0)).astype(ml_dtypes.bfloat16)
        bmv = d1[b].reshape(D, IJ).astype(ml_dtypes.float8_e4m3)
        mblk = mkv[b, isl:isl + 16].reshape(ROWS, IJ)
        m8v = (mblk.astype(np.float32) * np.float32(OMEGA)).astype(
            ml_dtypes.float8_e5m2)
        in_maps.append(
            {
                "a5": np.ascontiguousarray(a5),
                "bm": np.ascontiguousarray(bmv),
                "m8": np.ascontiguousarray(m8v),
                "idn": np.ascontiguousarray(idn),
            }
        )
        # per-unit linear sums (f64, from the same bf16 values the PE sees):
        # S_u = sum d', n1_u = mask popcount, for unit pid = g*2 + p
        a64 = a5.astype(np.float64)
        b64 = bmv.astype(np.float64)
        s_u = np.zeros(N_PAIRS)
        n1_u = np.zeros(N_PAIRS)
        for g in range(G):
            asum = a64[:, g * 128:(g + 1) * 128].sum(axis=1)
            for p in range(2):
                bsum = b64[:, p * PAIR:(p + 1) * PAIR].sum(axis=1)
                blk = mblk[g * 128:(g + 1) * 128, p * PAIR:(p + 1) * PAIR]
                s_u[g * 2 + p] = float(asum @ bsum)
                n1_u[g * 2 + p] = float(blk.sum(dtype=np.int64))
        side.append((s_u, n1_u))
    _cached["side"] = side
    return in_maps


def _run(in_maps, **kwargs):
    if "nc" not in _cached:
        _cached["nc"] = _build_program()
    return run_bass_kernel_spmd(_cached["nc"], in_maps, list(range(N_CORES)),
                                **kwargs)


def _combine(results):
    side = _cached["side"]
    NC_U = 128.0 * PAIR      # elements per unit
    # unit order in the device loop (ui) vs accumulator id (pid)
    units = [(g, p) for p in range(2) for g in range(G)]
    special_ui = len(units) - 1
    total = 0.0
    for r, (s_u, n1_u) in zip(results, side):
        acc = r["accs"].astype(np.float64)
        accA = acc[:, :N_PAIRS]
        accC = acc[:, N_PAIRS:]
        hinge1 = 0.0
        hinge2 = 0.0
        ui = 0
        while ui <= special_ui:
            g, p = units[ui]
            pid = g * 2 + p
            if ui == special_ui:
                # two-abs identity: contribution = A1/2 + 125*A2
                #   - 124.5*S + 112.5*n1 - 127375.5*n0
                a1 = accA[:, pid].sum()
                a2 = accC[:, ui].sum()
                n1 = n1_u[pid]
                n0 = NC_U - n1
                total += (0.5 * a1 + 125.0 * a2 - 124.5 * s_u[pid]
                          + 112.5 * n1 - (0.5 + 125.0 * (TH - 1.0)) * n0)
                ui += 1
                continue
            a_blk = accA[:, pid].sum()
            c_blk = accC[:, ui].sum()
            s_lin = s_u[pid] - OMEGA * n1_u[pid] - NC_U
            hinge1 += 0.5 * (s_lin + a_blk)
            hinge2 += a_blk - c_blk
            ui += 1
        total += hinge1 + 250.0 * hinge2
    return np.float32(total / (5.0 * B * IJ * IJ))


def kernel(descriptors_0, descriptors_1, similarity_mask):
    in_maps = _prep_inputs(descriptors_0, descriptors_1, similarity_mask)
    res = _run(in_maps)
    return _combine(res.results)
